# revision 12
# baseline (speedup 1.0000x reference)
"""Bahdanau-attention kernel for 8 TRN2 NeuronCores (batch-parallel SPMD).

Reference computation (per batch b of 8, per head h of 8, HID=128):
    kx = k[b] @ Wk + bk                    # (1024, 1024) -> heads on cols
    qs_h = (q[b] @ Wq + bq)_h @ w_q        # (1024,) per head (qx never needed)
    ks_h = kx_h @ w_k                      # (1024,)
    score = softmax(tanh(qs[:,None] + ks[None,:]), axis=-1)
    out_h = score @ kx_h
    out = concat_h(out_h) @ Wp + bp

Device-side layout choices:
  - k/q inputs host-transposed to [d_in, len] so projections contract d_in on
    partitions without on-device transposes.
  - Per-head score vectors fold into the K projection as 8 extra weight
    columns (U_k[:,h] = Wk_h @ w_k), so ks comes out of the same matmuls.
  - The Q projection only needs the 8 fused columns (qx is otherwise unused).
  - e = exp(tanh(.)) is built in [k_part, q_free] layout via the ACT engine's
    per-partition bias (ks) over a broadcast qs row; softmax denominators are
    M=1 ones-matmuls; AV product contracts k on partitions with kx as lhsT.
  - score goes to HBM as [head, k, q]; the host hands back a transposed view
    (the unshard step) to match the reference's [head*mb, q, k].
"""

import ml_dtypes
import numpy as np

import concourse.bass as bass
import concourse.mybir as mybir
from concourse.bass import ds, ts
from concourse.tile import TileContext
from concourse.vector_clock import ScopedClock

MB, LEN, D_IN, N_HEAD, HID, OUT_DIM = 8, 1024, 1024, 8, 128, 1024
P = 128
F32 = mybir.dt.float32
BF16 = mybir.dt.bfloat16
AF = mybir.ActivationFunctionType
ALU = mybir.AluOpType


COMPUTE_OPS_WAIT_LIMIT = 1


def _split_multiwaits(bir_bytes):
    """This walrus build encodes at most one sync-wait command per compute
    instruction.  Hoist extra on_wait entries onto fresh single-wait NOPs
    inserted just before the instruction on the same engine."""
    import json
    m = json.loads(bir_bytes)
    n_new = 0
    for fn in m.get("functions", []):
        def fix_blocks(blocks):
            nonlocal n_new
            for b in blocks:
                insts = b.get("instructions", [])
                out = []
                for inst in insts:
                    si = inst.get("sync_info") or {}
                    ow = si.get("on_wait") or []
                    if len(ow) > COMPUTE_OPS_WAIT_LIMIT:
                        keep = ow[-COMPUTE_OPS_WAIT_LIMIT:]
                        hoist = ow[:-COMPUTE_OPS_WAIT_LIMIT]
                        for k, w in enumerate(hoist):
                            n_new += 1
                            out.append({
                                "debug": inst.get("debug", 0),
                                "engine": inst["engine"],
                                "ins": [], "outs": [],
                                "name": f"{inst['name']}w{k}",
                                "opcode": "NoOp",
                                "sync_info": {"on_update": [], "on_wait": [w]},
                                "text_hint": "swsplit",
                            })
                        si["on_wait"] = keep
                    out.append(inst)
                b["instructions"] = out
                fix_blocks(b.get("blocks", []))
        fix_blocks(fn.get("blocks", []))
    return json.dumps(m).encode()


_SPLIT_INSTALLED = False


def _install_compile_patch():
    global _SPLIT_INSTALLED
    if _SPLIT_INSTALLED:
        return
    _SPLIT_INSTALLED = True
    import concourse.bass2jax as b2j
    import concourse.bass_utils as bu
    _orig = bu.compile_bir_kernel

    def patched(bir_json, tmpdir, neff_name="file.neff"):
        return _orig(_split_multiwaits(bir_json), tmpdir, neff_name)

    bu.compile_bir_kernel = patched
    b2j.compile_bir_kernel = patched


def _install_tail_fix():
    """This walrus build rejects Drain/CTRL instructions carrying more than one
    sync-wait command.  Replace TileContext's tail drain+barrier with one NOP
    per pending proc wait followed by sem-only barriers."""

    def _drain_and_barrier(self, tick_clock, wait_clock):
        nc = self.nc
        vec = tick_clock.global_clock
        for proc in range(len(vec)):
            tick = vec[proc]
            if tick > 0:
                nop = nc.sync.nop(nofuse=True, hint=f"tail_wait_p{proc}").ins
                sc = ScopedClock()
                sc.require_at_least(None, proc, tick)
                wait_clock.add_sem_waits(nop, sc)
        nc.sync.drain()
        nc.all_engine_barrier(sem_only=True)
        popped = nc._tile_sem_poison_stack.pop()
        assert popped is self._sem_poison
        nc.clear_and_free_semaphores(list(self.sems.allocated().values()))
        nc.all_engine_barrier(sem_only=True)

    TileContext._drain_and_barrier = _drain_and_barrier


_install_tail_fix()
_install_compile_patch()


def build(LEN=LEN, D=D_IN, H=N_HEAD, e_bufs=8, n_cores=MB):
    """Build the per-core Bass program.  LEN/D must be multiples of 128;
    OUT_DIM == D and head dim HID == 128 are assumed (OC == H)."""
    T = LEN // P      # len tiles
    C = D // P        # d_in contraction chunks
    NA = D + H        # augmented K-projection width
    OD = H * HID      # == D for the real problem
    OC = OD // P
    assert OC == H and HID == P
    NH = min(512, LEN)  # matmul/psum free-dim chunk
    NSPL = (LEN + NH - 1) // NH  # splits of LEN into NH chunks

    nc = bass.Bass(
        "TRN2", target_bir_lowering=False, debug=False,
        enable_asserts=False, num_devices=n_cores,
    )
    kT = nc.dram_tensor("kT", [D, LEN], BF16, kind="ExternalInput").ap()
    qT = nc.dram_tensor("qT", [D, LEN], BF16, kind="ExternalInput").ap()
    wk = nc.dram_tensor("wk", [D, NA], BF16, kind="ExternalInput").ap()
    bk = nc.dram_tensor("bk", [NA], F32, kind="ExternalInput").ap()
    uq = nc.dram_tensor("uq", [D, H], BF16, kind="ExternalInput").ap()
    cq = nc.dram_tensor("cq", [H], F32, kind="ExternalInput").ap()
    wp = nc.dram_tensor("wp", [D, OD], BF16, kind="ExternalInput").ap()
    ident_d = nc.dram_tensor("ident", [P, P], F32, kind="ExternalInput").ap()
    bp = nc.dram_tensor("bp", [OD], F32, kind="ExternalInput").ap()
    out = nc.dram_tensor("out", [LEN, OD], F32, kind="ExternalOutput").ap()
    scoreT = nc.dram_tensor("scoreT", [H, LEN, LEN], BF16, kind="ExternalOutput").ap()

    kT3 = kT.rearrange("(c p) l -> p c l", p=P)
    qT3 = qT.rearrange("(c p) l -> p c l", p=P)

    def mm(ps, lhsT, rhs, st, sp):
        nc.tensor.matmul(ps, lhsT, rhs, start=st, stop=sp)

    with TileContext(nc) as tc:
        with (
            tc.tile_pool(name="persist", bufs=1) as persist,
            tc.tile_pool(name="stream", bufs=2) as stream,
            tc.tile_pool(name="wts", bufs=1) as wts_pool,
            tc.tile_pool(name="small", bufs=1) as small,
            tc.tile_pool(name="work", bufs=2) as work,
            tc.tile_pool(name="rec", bufs=1) as rec_pool,
            tc.tile_pool(name="epool", bufs=e_bufs) as epool,
            tc.tile_pool(name="ps", bufs=2, space="PSUM") as ps_pool,
            tc.tile_pool(name="tp_ps", bufs=1, space="PSUM") as tp_ps,
            tc.tile_pool(name="dram", bufs=1, space="DRAM") as dram_pool,
        ):
            qsT_dram = dram_pool.tile([H, LEN], F32, tag="qsTd")
            rec_dram = dram_pool.tile([H, LEN], BF16, tag="recd")
            # ---- persistent tensors ----
            kx_sb = persist.tile([P, T, D], BF16, tag="kx")       # K proj output
            ks_sb = persist.tile([P, T, H], F32, tag="ks")        # fused score col
            oT_sb = persist.tile([P, H, LEN], BF16, tag="oT")     # attention outT

            ones_sb = small.tile([P, 1], BF16, tag="ones")
            nc.vector.memset(ones_sb[:], 1.0)
            ident = small.tile([P, P], F32, tag="ident")
            nc.sync.dma_start(ident[:], ident_d)
            bk_sb = small.tile([P, NA], F32, tag="bk")
            nc.sync.dma_start(bk_sb[:], bk[None, :].to_broadcast((P, NA)))
            cq_sb = small.tile([P, H], F32, tag="cq")
            nc.sync.dma_start(cq_sb[:], cq[None, :].to_broadcast((P, H)))
            bp_sb = small.tile([P, OD], F32, tag="bp")
            nc.sync.dma_start(bp_sb[:], bp[None, :].to_broadcast((P, OD)))

            # ---- phase A: kx = kT.T @ wk_aug + bk_aug ----
            wk_sb = wts_pool.tile([P, C, NA], BF16, tag="w")
            nc.sync.dma_start(wk_sb[:], wk.rearrange("(c p) n -> p c n", p=P))

            for t in range(T):
                kTt = stream.tile([P, C, P], BF16, tag="inT")
                nc.sync.dma_start(kTt[:], kT3[:, :, ts(t, P)])
                pss = [ps_pool.tile([P, NH], F32, tag="ab"[j], name=f"pss{t}_{j}") for j in range(NSPL)]
                ps3 = ps_pool.tile([P, NH], F32, tag="c", name=f"ps3k{t}")[:, :H]
                for c in range(C):
                    lhsT = kTt[:, c, :]
                    st, sp = c == 0, c == C - 1
                    for j in range(NSPL):
                        mm(pss[j][:], lhsT, wk_sb[:, c, ts(j, NH)], st, sp)
                    mm(ps3[:], lhsT, wk_sb[:, c, D:NA], st, sp)
                for j in range(NSPL):
                    nc.vector.tensor_tensor(
                        kx_sb[:, t, ts(j, NH)], pss[j][:], bk_sb[:, ts(j, NH)], ALU.add)
                nc.vector.tensor_tensor(
                    ks_sb[:, t, :], ps3[:], bk_sb[:, D:NA], ALU.add)

            # ---- phase B: qs = qT.T @ uq + cq, then transpose to [H, LEN] ----
            uq_sb = small.tile([P, C, H], BF16, tag="uq")
            nc.sync.dma_start(uq_sb[:], uq.rearrange("(c p) h -> p c h", p=P))
            qs_sb = small.tile([P, T, H], F32, tag="qs")
            for t in range(T):
                qTt = stream.tile([P, C, P], BF16, tag="inT")
                nc.sync.dma_start(qTt[:], qT3[:, :, ts(t, P)])
                ps3 = ps_pool.tile([P, NH], F32, tag="c", name=f"ps3q{t}")[:, :H]
                for c in range(C):
                    mm(ps3[:], qTt[:, c, :], uq_sb[:, c, :], c == 0, c == C - 1)
                nc.vector.tensor_tensor(qs_sb[:, t, :], ps3[:], cq_sb[:], ALU.add)
            qsT_sb = small.tile([H, LEN], F32, tag="qsT")
            for t in range(T):
                pst = tp_ps.tile([H, P], F32, tag="tp")
                nc.tensor.transpose(pst[:], qs_sb[:, t, :], ident[:])
                nc.vector.tensor_copy(out=qsT_sb[:, ts(t, P)], in_=pst[:])
            nc.sync.dma_start(qsT_dram[:], qsT_sb[:])

            # ---- phase C: per-head attention ----
            wp_sb = wts_pool.tile([P, OC, OD], BF16, tag="w")  # reuses wk slot
            nc.sync.dma_start(wp_sb[:], wp.rearrange("(c p) o -> p c o", p=P))

            for h in range(H):
                qs_bt = work.tile([P, LEN], F32, tag="qsb")
                nc.sync.dma_start(qs_bt[:], qsT_dram[h:h + 1, :].to_broadcast((P, LEN)))
                qs_b = qs_bt[:]
                ps_o = [ps_pool.tile([P, NH], F32, tag="ab"[j], name=f"pso{h}_{j}") for j in range(NSPL)]
                ps_r = [ps_pool.tile([P, NH], F32, tag="c", name=f"psr{h}_{j}")[0:1, :] for j in range(NSPL)]
                es = []
                for c in range(C):
                    s_t = work.tile([P, LEN], F32, tag="s")
                    nc.scalar.activation(
                        s_t[:], qs_b, AF.Tanh, bias=ks_sb[:, c, h:h + 1])
                    e_t = epool.tile([P, LEN], BF16, tag="e")
                    es.append(e_t)
                    nc.scalar.activation(e_t[:], s_t[:], AF.Exp)
                    lhsT = kx_sb[:, c, ds(h * P, P)]
                    st, sp = c == 0, c == C - 1
                    for j in range(NSPL):
                        mm(ps_o[j][:], lhsT, e_t[:, ts(j, NH)], st, sp)
                        mm(ps_r[j][:], ones_sb[:], e_t[:, ts(j, NH)], st, sp)
                recip = rec_pool.tile([1, LEN], F32, tag="recip")
                for j in range(NSPL):
                    nc.vector.reciprocal(recip[:, ts(j, NH)], ps_r[j][:])
                recip16 = rec_pool.tile([1, LEN], BF16, tag="recip16")
                nc.vector.tensor_copy(out=recip16[:], in_=recip[:])
                nc.sync.dma_start(rec_dram[h:h + 1, :], recip16[:])
                recip_b = work.tile([P, LEN], BF16, tag="recipb")
                nc.sync.dma_start(recip_b[:], rec_dram[h:h + 1, :].to_broadcast((P, LEN)))
                for j in range(NSPL):
                    nc.vector.tensor_tensor(
                        oT_sb[:, h, ts(j, NH)], ps_o[j][:], recip_b[:, ts(j, NH)], ALU.mult)
                for c in range(C):
                    nc.vector.tensor_tensor(es[c][:], es[c][:], recip_b[:], ALU.mult)
                    nc.sync.dma_start(scoreT[h, ds(c * P, P), :], es[c][:])

            # ---- final projection: out = oT.T @ wp + bp ----
            NHo = min(512, OD)
            NSPLo = (OD + NHo - 1) // NHo
            for t in range(T):
                psf = [ps_pool.tile([P, NHo], F32, tag="ab"[j], name=f"psf{t}_{j}") for j in range(NSPLo)]
                for c in range(OC):
                    lhsT = oT_sb[:, c, ts(t, P)]
                    st, sp = c == 0, c == OC - 1
                    for j in range(NSPLo):
                        mm(psf[j][:], lhsT, wp_sb[:, c, ts(j, NHo)], st, sp)
                fin = work.tile([P, OD], F32, tag="fin")
                for j in range(NSPLo):
                    nc.vector.tensor_tensor(
                        fin[:, ts(j, NHo)], psf[j][:], bp_sb[:, ts(j, NHo)], ALU.add)
                nc.sync.dma_start(out[ts(t, P), :], fin[:])

    return nc


def host_prep(k, q, Wk, bk, Wq, bq, w, Wp, bp, H=N_HEAD, HID_=HID):
    """Per-core input maps.  Folds the per-head score vectors into the
    projections (ks = kx_h @ w_k becomes extra columns of Wk)."""
    k = np.asarray(k, np.float32)
    q = np.asarray(q, np.float32)
    Wk = np.asarray(Wk, np.float32)
    bk = np.asarray(bk, np.float32)
    Wq = np.asarray(Wq, np.float32)
    bq = np.asarray(bq, np.float32)
    w = np.asarray(w, np.float32)
    Wp = np.asarray(Wp, np.float32)
    bp = np.asarray(bp, np.float32)
    D = Wk.shape[0]
    mb = k.shape[0]

    U_k = np.einsum("dhj,j->dh", Wk.reshape(D, H, HID_), w[:HID_]).astype(np.float32)
    c_k = (bk.reshape(H, HID_) @ w[:HID_]).astype(np.float32)
    U_q = np.einsum("dhj,j->dh", Wq.reshape(D, H, HID_), w[HID_:]).astype(np.float32)
    c_q = (bq.reshape(H, HID_) @ w[HID_:]).astype(np.float32)

    bf16 = ml_dtypes.bfloat16
    shared = {
        "wk": np.ascontiguousarray(np.concatenate([Wk, U_k], axis=1)).astype(bf16),
        "bk": np.ascontiguousarray(np.concatenate([bk, c_k])),
        "uq": np.ascontiguousarray(U_q).astype(bf16), "cq": c_q,
        "wp": Wp.astype(bf16), "bp": bp,
        "ident": np.eye(128, dtype=np.float32),
    }
    in_maps = []
    for b in range(mb):
        m = dict(shared)
        m["kT"] = np.ascontiguousarray(k[b].T).astype(ml_dtypes.bfloat16)
        m["qT"] = np.ascontiguousarray(q[b].T).astype(ml_dtypes.bfloat16)
        in_maps.append(m)
    return in_maps


_NC_CACHE = {}


def _install_ntff_shim():
    """This image lacks ``antenv.axon_hooks``; recreate it and register the
    ctypes NTFF hook against the injected libaxon_pjrt.so, and skip the S3
    artifact upload (no bucket access here)."""
    import sys, types
    try:
        from antenv.axon_hooks import get_axon_ntff_profile_hook  # noqa: F401
        return
    except ImportError:
        pass
    import antenv
    mod = types.ModuleType("antenv.axon_hooks")
    _h = [None]
    mod.set_axon_ntff_profile_hook = lambda h: _h.__setitem__(0, h)
    mod.get_axon_ntff_profile_hook = lambda: _h[0]
    sys.modules["antenv.axon_hooks"] = mod
    antenv.axon_hooks = mod
    try:
        from trn_agent_boot.trn_boot import _ntff_profile_via_ctypes
        mod.set_axon_ntff_profile_hook(
            _ntff_profile_via_ctypes("/opt/axon/libaxon_pjrt.so"))
    except Exception:
        pass
    import concourse.bass_utils as bu
    bu.upload_artifacts = lambda tmpdir: str(tmpdir)


def kernel(k, q, Wk, bk, Wq, bq, w, Wp, bp, _trace=False):
    from concourse.bass_utils import run_bass_kernel_spmd

    if _trace:
        _install_ntff_shim()

    in_maps = host_prep(k, q, Wk, bk, Wq, bq, w, Wp, bp)
    if "nc" not in _NC_CACHE:
        _NC_CACHE["nc"] = build()
    nc = _NC_CACHE["nc"]
    kwargs = {}
    if _trace:
        kwargs = dict(trace=True, trace_cores=[0])
    res = run_bass_kernel_spmd(nc, in_maps, core_ids=list(range(MB)), **kwargs)

    out = np.stack([res.results[b]["out"] for b in range(MB)])
    # scoreT per core: [H, k, q]; reference wants score[h*MB+b, q, k].
    sT = np.stack([res.results[b]["scoreT"] for b in range(MB)])  # [b, h, k, q]
    score = sT.transpose(1, 0, 3, 2).reshape(N_HEAD * MB, LEN, LEN).astype(np.float32)
    if _trace:
        kernel._last_results = res
    return out, score


# revision 13
# speedup vs baseline: 1.3786x; 1.3786x over previous
"""Bahdanau-attention kernel for 8 TRN2 NeuronCores (batch-parallel SPMD).

Reference computation (per batch b of 8, per head h of 8, HID=128):
    kx = k[b] @ Wk + bk                    # (1024, 1024) -> heads on cols
    qs_h = (q[b] @ Wq + bq)_h @ w_q        # (1024,) per head (qx never needed)
    ks_h = kx_h @ w_k                      # (1024,)
    score = softmax(tanh(qs[:,None] + ks[None,:]), axis=-1)
    out_h = score @ kx_h
    out = concat_h(out_h) @ Wp + bp

Device-side layout choices:
  - k/q inputs host-transposed to [d_in, len] so projections contract d_in on
    partitions without on-device transposes.
  - Per-head score vectors fold into the K projection as 8 extra weight
    columns (U_k[:,h] = Wk_h @ w_k), so ks comes out of the same matmuls.
  - The Q projection only needs the 8 fused columns (qx is otherwise unused).
  - e = exp(tanh(.)) is built in [k_part, q_free] layout via the ACT engine's
    per-partition bias (ks) over a broadcast qs row; softmax denominators are
    M=1 ones-matmuls; AV product contracts k on partitions with kx as lhsT.
  - score goes to HBM as [head, k, q]; the host hands back a transposed view
    (the unshard step) to match the reference's [head*mb, q, k].
"""

import ml_dtypes
import numpy as np

import concourse.bass as bass
import concourse.mybir as mybir
from concourse.bass import ds, ts
from concourse.tile import TileContext
from concourse.vector_clock import ScopedClock

MB, LEN, D_IN, N_HEAD, HID, OUT_DIM = 8, 1024, 1024, 8, 128, 1024
P = 128
F32 = mybir.dt.float32
BF16 = mybir.dt.bfloat16
AF = mybir.ActivationFunctionType
ALU = mybir.AluOpType


COMPUTE_OPS_WAIT_LIMIT = 1


def _split_multiwaits(bir_bytes):
    """This walrus build encodes at most one sync-wait command per compute
    instruction.  Hoist extra on_wait entries onto fresh single-wait NOPs
    inserted just before the instruction on the same engine."""
    import json
    m = json.loads(bir_bytes)
    n_new = 0
    for fn in m.get("functions", []):
        def fix_blocks(blocks):
            nonlocal n_new
            for b in blocks:
                insts = b.get("instructions", [])
                out = []
                for inst in insts:
                    si = inst.get("sync_info") or {}
                    ow = si.get("on_wait") or []
                    if len(ow) > COMPUTE_OPS_WAIT_LIMIT:
                        keep = ow[-COMPUTE_OPS_WAIT_LIMIT:]
                        hoist = ow[:-COMPUTE_OPS_WAIT_LIMIT]
                        for k, w in enumerate(hoist):
                            n_new += 1
                            out.append({
                                "debug": inst.get("debug", 0),
                                "engine": inst["engine"],
                                "ins": [], "outs": [],
                                "name": f"{inst['name']}w{k}",
                                "opcode": "NoOp",
                                "sync_info": {"on_update": [], "on_wait": [w]},
                                "text_hint": "swsplit",
                            })
                        si["on_wait"] = keep
                    out.append(inst)
                b["instructions"] = out
                fix_blocks(b.get("blocks", []))
        fix_blocks(fn.get("blocks", []))
    return json.dumps(m).encode()


_SPLIT_INSTALLED = False


def _install_compile_patch():
    global _SPLIT_INSTALLED
    if _SPLIT_INSTALLED:
        return
    _SPLIT_INSTALLED = True
    import concourse.bass2jax as b2j
    import concourse.bass_utils as bu
    _orig = bu.compile_bir_kernel

    def patched(bir_json, tmpdir, neff_name="file.neff"):
        return _orig(_split_multiwaits(bir_json), tmpdir, neff_name)

    bu.compile_bir_kernel = patched
    b2j.compile_bir_kernel = patched


def _install_tail_fix():
    """This walrus build rejects Drain/CTRL instructions carrying more than one
    sync-wait command.  Replace TileContext's tail drain+barrier with one NOP
    per pending proc wait followed by sem-only barriers."""

    def _drain_and_barrier(self, tick_clock, wait_clock):
        nc = self.nc
        vec = tick_clock.global_clock
        for proc in range(len(vec)):
            tick = vec[proc]
            if tick > 0:
                nop = nc.sync.nop(nofuse=True, hint=f"tail_wait_p{proc}").ins
                sc = ScopedClock()
                sc.require_at_least(None, proc, tick)
                wait_clock.add_sem_waits(nop, sc)
        nc.sync.drain()
        nc.all_engine_barrier(sem_only=True)
        popped = nc._tile_sem_poison_stack.pop()
        assert popped is self._sem_poison
        nc.clear_and_free_semaphores(list(self.sems.allocated().values()))
        nc.all_engine_barrier(sem_only=True)

    TileContext._drain_and_barrier = _drain_and_barrier


_install_tail_fix()
_install_compile_patch()


def build(LEN=LEN, D=D_IN, H=N_HEAD, e_bufs=8, n_cores=MB):
    """Build the per-core Bass program.  LEN/D must be multiples of 128;
    OUT_DIM == D and head dim HID == 128 are assumed (OC == H)."""
    T = LEN // P      # len tiles
    C = D // P        # d_in contraction chunks
    NA = D + H        # augmented K-projection width
    OD = H * HID      # == D for the real problem
    OC = OD // P
    assert OC == H and HID == P
    NH = min(512, LEN)  # matmul/psum free-dim chunk
    NSPL = (LEN + NH - 1) // NH  # splits of LEN into NH chunks

    nc = bass.Bass(
        "TRN2", target_bir_lowering=False, debug=False,
        enable_asserts=False, num_devices=n_cores,
    )
    kT = nc.dram_tensor("kT", [D, LEN], BF16, kind="ExternalInput").ap()
    qT = nc.dram_tensor("qT", [D, LEN], BF16, kind="ExternalInput").ap()
    wk = nc.dram_tensor("wk", [D, NA], BF16, kind="ExternalInput").ap()
    bk = nc.dram_tensor("bk", [NA], F32, kind="ExternalInput").ap()
    uq = nc.dram_tensor("uq", [D, H], BF16, kind="ExternalInput").ap()
    cq = nc.dram_tensor("cq", [H], F32, kind="ExternalInput").ap()
    wp = nc.dram_tensor("wp", [D, OD], BF16, kind="ExternalInput").ap()
    ident_d = nc.dram_tensor("ident", [P, P], F32, kind="ExternalInput").ap()
    bp = nc.dram_tensor("bp", [OD], F32, kind="ExternalInput").ap()
    out = nc.dram_tensor("out", [LEN, OD], F32, kind="ExternalOutput").ap()
    scoreT = nc.dram_tensor("scoreT", [H, LEN, LEN], BF16, kind="ExternalOutput").ap()

    kT3 = kT.rearrange("(c p) l -> p c l", p=P)
    qT3 = qT.rearrange("(c p) l -> p c l", p=P)

    def mm(ps, lhsT, rhs, st, sp):
        nc.tensor.matmul(ps, lhsT, rhs, start=st, stop=sp)

    with TileContext(nc) as tc:
        G = 4 if C % 4 == 0 else C  # exp/e-tile chunk grouping
        NG = C // G
        with (
            tc.tile_pool(name="persist", bufs=1) as persist,
            tc.tile_pool(name="stream", bufs=2) as stream,
            tc.tile_pool(name="wts", bufs=1) as wts_pool,
            tc.tile_pool(name="small", bufs=1) as small,
            tc.tile_pool(name="work", bufs=2) as work,
            tc.tile_pool(name="rec", bufs=2) as rec_pool,
            tc.tile_pool(name="epool", bufs=2 * NG + 1) as epool,
            tc.tile_pool(name="ps", bufs=2, space="PSUM") as ps_pool,
            tc.tile_pool(name="dram", bufs=1, space="DRAM") as dram_pool,
        ):
            qsT_dram = dram_pool.tile([H, LEN], F32, tag="qsTd")
            rec_dram = dram_pool.tile([H, LEN], BF16, tag="recd")

            # ---- persistent tensors ----
            kx_t = [persist.tile([P, D], BF16, tag=f"kx{t}", name=f"kx_t{t}") for t in range(T)]
            ks_t = [persist.tile([P, H], F32, tag=f"ks{t}", name=f"ks_t{t}") for t in range(T)]
            oT_sb = persist.tile([P, H, LEN], BF16, tag="oT")
            ones_sb = small.tile([P, 1], BF16, tag="ones")
            nc.vector.memset(ones_sb[:], 1.0)
            ident = small.tile([P, P], F32, tag="ident")
            nc.sync.dma_start(ident[:], ident_d)
            bk_sb = small.tile([P, NA], F32, tag="bk")
            nc.sync.dma_start(bk_sb[:], bk[None, :].to_broadcast((P, NA)))
            cq_sb = small.tile([P, H], F32, tag="cq")
            nc.sync.dma_start(cq_sb[:], cq[None, :].to_broadcast((P, H)))
            bp_sb = small.tile([P, OD], F32, tag="bp")
            nc.sync.dma_start(bp_sb[:], bp[None, :].to_broadcast((P, OD)))

            # ---- phase B: qs = qT.T @ uq + cq, transpose to [H, LEN], to DRAM ----
            uq_sb = small.tile([P, C, H], BF16, tag="uq")
            nc.sync.dma_start(uq_sb[:], uq.rearrange("(c p) h -> p c h", p=P))
            qs_sb = small.tile([P, T, H], F32, tag="qs")
            for t in range(T):
                qTt = stream.tile([P, C, P], BF16, tag="inT")
                nc.sync.dma_start(qTt[:], qT3[:, :, ts(t, P)])
                ps3 = ps_pool.tile([P, NH], F32, tag="c", name=f"ps3q{t}")[:, :H]
                for c in range(C):
                    mm(ps3[:], qTt[:, c, :], uq_sb[:, c, :], c == 0, c == C - 1)
                nc.vector.tensor_tensor(qs_sb[:, t, :], ps3[:], cq_sb[:], ALU.add)
            qsT_sb = small.tile([H, LEN], F32, tag="qsT")
            for t in range(T):
                pst = ps_pool.tile([P, NH], F32, tag="c", name=f"pst{t}")[:H, :P]
                nc.tensor.transpose(pst[:], qs_sb[:, t, :], ident[:])
                nc.vector.tensor_copy(out=qsT_sb[:, ts(t, P)], in_=pst[:])
            nc.sync.dma_start(qsT_dram[:], qsT_sb[:])

            # ---- phase A: kx = kT.T @ wk_aug + bk_aug (per-t tiles) ----
            wk_sb = wts_pool.tile([P, C, NA], BF16, tag="w")
            nc.sync.dma_start(wk_sb[:], wk.rearrange("(c p) n -> p c n", p=P))
            for t in range(T):
                kTt = stream.tile([P, C, P], BF16, tag="inT")
                nc.sync.dma_start(kTt[:], kT3[:, :, ts(t, P)])
                pss = [ps_pool.tile([P, NH], F32, tag="ab"[j], name=f"pss{t}_{j}") for j in range(NSPL)]
                ps3 = ps_pool.tile([P, NH], F32, tag="c", name=f"ps3k{t}")[:, :H]
                for c in range(C):
                    lhsT = kTt[:, c, :]
                    st, sp = c == 0, c == C - 1
                    for j in range(NSPL):
                        mm(pss[j][:], lhsT, wk_sb[:, c, ts(j, NH)], st, sp)
                    mm(ps3[:], lhsT, wk_sb[:, c, D:NA], st, sp)
                for j in range(NSPL):
                    nc.vector.tensor_tensor(
                        kx_t[t][:, ts(j, NH)], pss[j][:], bk_sb[:, ts(j, NH)], ALU.add)
                nc.vector.tensor_tensor(ks_t[t][:, :], ps3[:], bk_sb[:, D:NA], ALU.add)

            # ---- phase C: per-head attention ----
            wp_sb = wts_pool.tile([P, OC, OD], BF16, tag="wp")
            nc.sync.dma_start(wp_sb[:], wp.rearrange("(c p) o -> p c o", p=P))

            for h in range(H):
                qs_bt = work.tile([P, LEN], F32, tag="qsb")
                nc.sync.dma_start(qs_bt[:], qsT_dram[h:h + 1, :].to_broadcast((P, LEN)))
                ps_o = [ps_pool.tile([P, NH], F32, tag="ab"[j], name=f"pso{h}_{j}") for j in range(NSPL)]
                ps_r = [ps_pool.tile([P, NH], F32, tag="r", name=f"psr{h}_{j}")[0:1, :] for j in range(NSPL)]
                egs = []
                for g in range(NG):
                    s_t = work.tile([P, G, LEN], F32, tag="s")
                    e_t = epool.tile([P, G, LEN], BF16, tag="e")
                    egs.append(e_t)
                    for i in range(G):
                        c = g * G + i
                        nc.scalar.activation(
                            s_t[:, i, :], qs_bt[:], AF.Tanh, bias=ks_t[c][:, h:h + 1])
                    nc.scalar.activation(e_t[:], s_t[:], AF.Exp)
                    for i in range(G):
                        c = g * G + i
                        lhsT = kx_t[c][:, ds(h * P, P)]
                        st, sp = c == 0, c == C - 1
                        for j in range(NSPL):
                            mm(ps_o[j][:], lhsT, e_t[:, i, ts(j, NH)], st, sp)
                        for j in range(NSPL):
                            mm(ps_r[j][:], ones_sb[:], e_t[:, i, ts(j, NH)], st, sp)
                # free rowsum psums fast, then recip chain off the hot path
                rs_sb = rec_pool.tile([1, LEN], F32, tag="rs")
                for j in range(NSPL):
                    nc.vector.tensor_copy(out=rs_sb[:, ts(j, NH)], in_=ps_r[j][:])
                recip_f = rec_pool.tile([1, LEN], F32, tag="recipf")
                nc.vector.reciprocal(recip_f[:], rs_sb[:])
                recip16 = rec_pool.tile([1, LEN], BF16, tag="recip16")
                nc.vector.tensor_copy(out=recip16[:], in_=recip_f[:])
                nc.sync.dma_start(rec_dram[h:h + 1, :], recip16[:])
                recip_b = work.tile([P, LEN], BF16, tag="recipb")
                nc.sync.dma_start(recip_b[:], rec_dram[h:h + 1, :].to_broadcast((P, LEN)))
                for j in range(NSPL):
                    nc.vector.tensor_tensor(
                        oT_sb[:, h, ts(j, NH)], ps_o[j][:], recip_b[:, ts(j, NH)], ALU.mult)
                for g in range(NG):
                    nc.vector.tensor_tensor(
                        egs[g][:], egs[g][:],
                        recip_b[:, None, :].to_broadcast((P, G, LEN)), ALU.mult)
                    nc.sync.dma_start(
                        scoreT[h, ds(g * G * P, G * P), :].rearrange(
                            "(i p) q -> p i q", p=P),
                        egs[g][:])

            # ---- final projection: out = oT.T @ wp + bp ----
            NHo = min(512, OD)
            NSPLo = (OD + NHo - 1) // NHo
            for t in range(T):
                psf = [ps_pool.tile([P, NHo], F32, tag="ab"[j], name=f"psf{t}_{j}") for j in range(NSPLo)]
                for c in range(OC):
                    lhsT = oT_sb[:, c, ts(t, P)]
                    st, sp = c == 0, c == OC - 1
                    for j in range(NSPLo):
                        mm(psf[j][:], lhsT, wp_sb[:, c, ts(j, NHo)], st, sp)
                fin = work.tile([P, OD], F32, tag="fin")
                for j in range(NSPLo):
                    nc.vector.tensor_tensor(
                        fin[:, ts(j, NHo)], psf[j][:], bp_sb[:, ts(j, NHo)], ALU.add)
                nc.sync.dma_start(out[ts(t, P), :], fin[:])

    return nc


def host_prep(k, q, Wk, bk, Wq, bq, w, Wp, bp, H=N_HEAD, HID_=HID):
    """Per-core input maps.  Folds the per-head score vectors into the
    projections (ks = kx_h @ w_k becomes extra columns of Wk)."""
    k = np.asarray(k, np.float32)
    q = np.asarray(q, np.float32)
    Wk = np.asarray(Wk, np.float32)
    bk = np.asarray(bk, np.float32)
    Wq = np.asarray(Wq, np.float32)
    bq = np.asarray(bq, np.float32)
    w = np.asarray(w, np.float32)
    Wp = np.asarray(Wp, np.float32)
    bp = np.asarray(bp, np.float32)
    D = Wk.shape[0]
    mb = k.shape[0]

    U_k = np.einsum("dhj,j->dh", Wk.reshape(D, H, HID_), w[:HID_]).astype(np.float32)
    c_k = (bk.reshape(H, HID_) @ w[:HID_]).astype(np.float32)
    U_q = np.einsum("dhj,j->dh", Wq.reshape(D, H, HID_), w[HID_:]).astype(np.float32)
    c_q = (bq.reshape(H, HID_) @ w[HID_:]).astype(np.float32)

    bf16 = ml_dtypes.bfloat16
    shared = {
        "wk": np.ascontiguousarray(np.concatenate([Wk, U_k], axis=1)).astype(bf16),
        "bk": np.ascontiguousarray(np.concatenate([bk, c_k])),
        "uq": np.ascontiguousarray(U_q).astype(bf16), "cq": c_q,
        "wp": Wp.astype(bf16), "bp": bp,
        "ident": np.eye(128, dtype=np.float32),
    }
    in_maps = []
    for b in range(mb):
        m = dict(shared)
        m["kT"] = np.ascontiguousarray(k[b].T).astype(ml_dtypes.bfloat16)
        m["qT"] = np.ascontiguousarray(q[b].T).astype(ml_dtypes.bfloat16)
        in_maps.append(m)
    return in_maps


_NC_CACHE = {}


def _install_ntff_shim():
    """This image lacks ``antenv.axon_hooks``; recreate it and register the
    ctypes NTFF hook against the injected libaxon_pjrt.so, and skip the S3
    artifact upload (no bucket access here)."""
    import sys, types
    try:
        from antenv.axon_hooks import get_axon_ntff_profile_hook  # noqa: F401
        return
    except ImportError:
        pass
    import antenv
    mod = types.ModuleType("antenv.axon_hooks")
    _h = [None]
    mod.set_axon_ntff_profile_hook = lambda h: _h.__setitem__(0, h)
    mod.get_axon_ntff_profile_hook = lambda: _h[0]
    sys.modules["antenv.axon_hooks"] = mod
    antenv.axon_hooks = mod
    try:
        from trn_agent_boot.trn_boot import _ntff_profile_via_ctypes
        mod.set_axon_ntff_profile_hook(
            _ntff_profile_via_ctypes("/opt/axon/libaxon_pjrt.so"))
    except Exception:
        pass
    import concourse.bass_utils as bu
    bu.upload_artifacts = lambda tmpdir: str(tmpdir)


def kernel(k, q, Wk, bk, Wq, bq, w, Wp, bp, _trace=False):
    from concourse.bass_utils import run_bass_kernel_spmd

    if _trace:
        _install_ntff_shim()

    in_maps = host_prep(k, q, Wk, bk, Wq, bq, w, Wp, bp)
    if "nc" not in _NC_CACHE:
        _NC_CACHE["nc"] = build()
    nc = _NC_CACHE["nc"]
    kwargs = {}
    if _trace:
        kwargs = dict(trace=True, trace_cores=[0])
    res = run_bass_kernel_spmd(nc, in_maps, core_ids=list(range(MB)), **kwargs)

    out = np.stack([res.results[b]["out"] for b in range(MB)])
    # scoreT per core: [H, k, q]; reference wants score[h*MB+b, q, k].
    sT = np.stack([res.results[b]["scoreT"] for b in range(MB)])  # [b, h, k, q]
    score = sT.transpose(1, 0, 3, 2).reshape(N_HEAD * MB, LEN, LEN).astype(np.float32)
    if _trace:
        kernel._last_results = res
    return out, score


# revision 15
# speedup vs baseline: 1.3878x; 1.0067x over previous
"""Bahdanau-attention kernel for 8 TRN2 NeuronCores (batch-parallel SPMD).

Reference computation (per batch b of 8, per head h of 8, HID=128):
    kx = k[b] @ Wk + bk                    # (1024, 1024) -> heads on cols
    qs_h = (q[b] @ Wq + bq)_h @ w_q        # (1024,) per head (qx never needed)
    ks_h = kx_h @ w_k                      # (1024,)
    score = softmax(tanh(qs[:,None] + ks[None,:]), axis=-1)
    out_h = score @ kx_h
    out = concat_h(out_h) @ Wp + bp

Device-side layout choices:
  - k/q inputs host-transposed to [d_in, len] so projections contract d_in on
    partitions without on-device transposes.
  - Per-head score vectors fold into the K projection as 8 extra weight
    columns (U_k[:,h] = Wk_h @ w_k), so ks comes out of the same matmuls.
  - The Q projection only needs the 8 fused columns (qx is otherwise unused).
  - e = exp(tanh(.)) is built in [k_part, q_free] layout via the ACT engine's
    per-partition bias (ks) over a broadcast qs row; softmax denominators are
    M=1 ones-matmuls; AV product contracts k on partitions with kx as lhsT.
  - score goes to HBM as [head, k, q]; the host hands back a transposed view
    (the unshard step) to match the reference's [head*mb, q, k].
"""

import ml_dtypes
import numpy as np

import concourse.bass as bass
import concourse.mybir as mybir
from concourse.bass import ds, ts
from concourse.tile import TileContext
from concourse.vector_clock import ScopedClock

MB, LEN, D_IN, N_HEAD, HID, OUT_DIM = 8, 1024, 1024, 8, 128, 1024
P = 128
F32 = mybir.dt.float32
BF16 = mybir.dt.bfloat16
AF = mybir.ActivationFunctionType
ALU = mybir.AluOpType


COMPUTE_OPS_WAIT_LIMIT = 1


def _split_multiwaits(bir_bytes):
    """This walrus build encodes at most one sync-wait command per compute
    instruction.  Hoist extra on_wait entries onto fresh single-wait NOPs
    inserted just before the instruction on the same engine."""
    import json
    m = json.loads(bir_bytes)
    n_new = 0
    for fn in m.get("functions", []):
        def fix_blocks(blocks):
            nonlocal n_new
            for b in blocks:
                insts = b.get("instructions", [])
                out = []
                for inst in insts:
                    si = inst.get("sync_info") or {}
                    ow = si.get("on_wait") or []
                    if len(ow) > COMPUTE_OPS_WAIT_LIMIT:
                        keep = ow[-COMPUTE_OPS_WAIT_LIMIT:]
                        hoist = ow[:-COMPUTE_OPS_WAIT_LIMIT]
                        for k, w in enumerate(hoist):
                            n_new += 1
                            out.append({
                                "debug": inst.get("debug", 0),
                                "engine": inst["engine"],
                                "ins": [], "outs": [],
                                "name": f"{inst['name']}w{k}",
                                "opcode": "NoOp",
                                "sync_info": {"on_update": [], "on_wait": [w]},
                                "text_hint": "swsplit",
                            })
                        si["on_wait"] = keep
                    out.append(inst)
                b["instructions"] = out
                fix_blocks(b.get("blocks", []))
        fix_blocks(fn.get("blocks", []))
    return json.dumps(m).encode()


_SPLIT_INSTALLED = False


def _install_compile_patch():
    global _SPLIT_INSTALLED
    if _SPLIT_INSTALLED:
        return
    _SPLIT_INSTALLED = True
    import concourse.bass2jax as b2j
    import concourse.bass_utils as bu
    _orig = bu.compile_bir_kernel

    def patched(bir_json, tmpdir, neff_name="file.neff"):
        return _orig(_split_multiwaits(bir_json), tmpdir, neff_name)

    bu.compile_bir_kernel = patched
    b2j.compile_bir_kernel = patched


def _install_tail_fix():
    """This walrus build rejects Drain/CTRL instructions carrying more than one
    sync-wait command.  Replace TileContext's tail drain+barrier with one NOP
    per pending proc wait followed by sem-only barriers."""

    def _drain_and_barrier(self, tick_clock, wait_clock):
        nc = self.nc
        vec = tick_clock.global_clock
        for proc in range(len(vec)):
            tick = vec[proc]
            if tick > 0:
                nop = nc.sync.nop(nofuse=True, hint=f"tail_wait_p{proc}").ins
                sc = ScopedClock()
                sc.require_at_least(None, proc, tick)
                wait_clock.add_sem_waits(nop, sc)
        nc.sync.drain()
        nc.all_engine_barrier(sem_only=True)
        popped = nc._tile_sem_poison_stack.pop()
        assert popped is self._sem_poison
        nc.clear_and_free_semaphores(list(self.sems.allocated().values()))
        nc.all_engine_barrier(sem_only=True)

    TileContext._drain_and_barrier = _drain_and_barrier


_install_tail_fix()
_install_compile_patch()


def build(LEN=LEN, D=D_IN, H=N_HEAD, e_bufs=8, n_cores=MB):
    """Build the per-core Bass program.  LEN/D must be multiples of 128;
    OUT_DIM == D and head dim HID == 128 are assumed (OC == H)."""
    T = LEN // P      # len tiles
    C = D // P        # d_in contraction chunks
    NA = D + H        # augmented K-projection width
    OD = H * HID      # == D for the real problem
    OC = OD // P
    assert OC == H and HID == P
    NH = min(512, LEN)  # matmul/psum free-dim chunk
    NSPL = (LEN + NH - 1) // NH  # splits of LEN into NH chunks

    nc = bass.Bass(
        "TRN2", target_bir_lowering=False, debug=False,
        enable_asserts=False, num_devices=n_cores,
    )
    kT = nc.dram_tensor("kT", [D, LEN], BF16, kind="ExternalInput").ap()
    qT = nc.dram_tensor("qT", [D, LEN], BF16, kind="ExternalInput").ap()
    wk = nc.dram_tensor("wk", [D, NA], BF16, kind="ExternalInput").ap()
    bk = nc.dram_tensor("bk", [NA], F32, kind="ExternalInput").ap()
    uq = nc.dram_tensor("uq", [D, H], BF16, kind="ExternalInput").ap()
    cq = nc.dram_tensor("cq", [H], F32, kind="ExternalInput").ap()
    wp = nc.dram_tensor("wp", [D, OD], BF16, kind="ExternalInput").ap()
    ident_d = nc.dram_tensor("ident", [P, P], F32, kind="ExternalInput").ap()
    bp = nc.dram_tensor("bp", [OD], F32, kind="ExternalInput").ap()
    out = nc.dram_tensor("out", [LEN, OD], F32, kind="ExternalOutput").ap()
    scoreT = nc.dram_tensor("scoreT", [H, LEN, LEN], BF16, kind="ExternalOutput").ap()

    kT3 = kT.rearrange("(c p) l -> p c l", p=P)
    qT3 = qT.rearrange("(c p) l -> p c l", p=P)

    def mm(ps, lhsT, rhs, st, sp):
        nc.tensor.matmul(ps, lhsT, rhs, start=st, stop=sp)

    with TileContext(nc) as tc:
        G = 4 if C % 4 == 0 else C  # exp/e-tile chunk grouping
        NG = C // G
        with (
            tc.tile_pool(name="persist", bufs=1) as persist,
            tc.tile_pool(name="stream", bufs=2) as stream,
            tc.tile_pool(name="wts", bufs=1) as wts_pool,
            tc.tile_pool(name="small", bufs=1) as small,
            tc.tile_pool(name="work", bufs=2) as work,
            tc.tile_pool(name="rec", bufs=2) as rec_pool,
            tc.tile_pool(name="epool", bufs=2 * NG + 1) as epool,
            tc.tile_pool(name="ps", bufs=2, space="PSUM") as ps_pool,
            tc.tile_pool(name="dram", bufs=1, space="DRAM") as dram_pool,
        ):
            qsT_dram = dram_pool.tile([H, LEN], F32, tag="qsTd")
            rec_dram = dram_pool.tile([H, LEN], BF16, tag="recd")

            # ---- persistent tensors ----
            kx_t = [persist.tile([P, D], BF16, tag=f"kx{t}", name=f"kx_t{t}") for t in range(T)]
            ks_t = [persist.tile([P, H], F32, tag=f"ks{t}", name=f"ks_t{t}") for t in range(T)]
            oT_sb = persist.tile([P, H, LEN], BF16, tag="oT")
            ones_sb = small.tile([P, 1], BF16, tag="ones")
            nc.vector.memset(ones_sb[:], 1.0)
            ident = small.tile([P, P], F32, tag="ident")
            nc.sync.dma_start(ident[:], ident_d)
            bk_sb = small.tile([P, NA], F32, tag="bk")
            nc.sync.dma_start(bk_sb[:], bk[None, :].to_broadcast((P, NA)))
            cq_sb = small.tile([P, H], F32, tag="cq")
            nc.sync.dma_start(cq_sb[:], cq[None, :].to_broadcast((P, H)))
            bp_sb = small.tile([P, OD], F32, tag="bp")
            nc.sync.dma_start(bp_sb[:], bp[None, :].to_broadcast((P, OD)))

            # ---- phase B: qs = qT.T @ uq + cq, transpose to [H, LEN], to DRAM ----
            uq_sb = small.tile([P, C, H], BF16, tag="uq")
            nc.sync.dma_start(uq_sb[:], uq.rearrange("(c p) h -> p c h", p=P))
            qs_sb = small.tile([P, T, H], F32, tag="qs")
            for t in range(T):
                qTt = stream.tile([P, C, P], BF16, tag="inT")
                nc.sync.dma_start(qTt[:], qT3[:, :, ts(t, P)])
                ps3 = ps_pool.tile([P, NH], F32, tag="c", name=f"ps3q{t}")[:, :H]
                for c in range(C):
                    mm(ps3[:], qTt[:, c, :], uq_sb[:, c, :], c == 0, c == C - 1)
                nc.vector.tensor_tensor(qs_sb[:, t, :], ps3[:], cq_sb[:], ALU.add)
            qsT_sb = small.tile([H, LEN], F32, tag="qsT")
            for t in range(T):
                pst = ps_pool.tile([P, NH], F32, tag="c", name=f"pst{t}")[:H, :P]
                nc.tensor.transpose(pst[:], qs_sb[:, t, :], ident[:])
                nc.vector.tensor_copy(out=qsT_sb[:, ts(t, P)], in_=pst[:])
            nc.sync.dma_start(qsT_dram[:], qsT_sb[:])

            # ---- phase A: kx = kT.T @ wk_aug + bk_aug (per-t tiles) ----
            wk_sb = wts_pool.tile([P, C, NA], BF16, tag="w")
            wk3 = wk.rearrange("(c p) n -> p c n", p=P)
            for c in range(C):
                nc.sync.dma_start(wk_sb[:, c, :], wk3[:, c, :])
            for t in range(T):
                kTt = stream.tile([P, C, P], BF16, tag="inT")
                nc.sync.dma_start(kTt[:], kT3[:, :, ts(t, P)])
                pss = [ps_pool.tile([P, NH], F32, tag="ab"[j], name=f"pss{t}_{j}") for j in range(NSPL)]
                ps3 = ps_pool.tile([P, NH], F32, tag="c", name=f"ps3k{t}")[:, :H]
                for c in range(C):
                    lhsT = kTt[:, c, :]
                    st, sp = c == 0, c == C - 1
                    for j in range(NSPL):
                        mm(pss[j][:], lhsT, wk_sb[:, c, ts(j, NH)], st, sp)
                    mm(ps3[:], lhsT, wk_sb[:, c, D:NA], st, sp)
                for j in range(NSPL):
                    nc.vector.tensor_tensor(
                        kx_t[t][:, ts(j, NH)], pss[j][:], bk_sb[:, ts(j, NH)], ALU.add)
                nc.vector.tensor_tensor(ks_t[t][:, :], ps3[:], bk_sb[:, D:NA], ALU.add)

            # ---- phase C: per-head attention ----
            wp_sb = wts_pool.tile([P, OC, OD], BF16, tag="wp")
            wp3 = wp.rearrange("(c p) o -> p c o", p=P)
            for c in range(OC):
                nc.sync.dma_start(wp_sb[:, c, :], wp3[:, c, :])

            for h in range(H):
                qs_bt = work.tile([P, LEN], F32, tag="qsb")
                nc.sync.dma_start(qs_bt[:], qsT_dram[h:h + 1, :].to_broadcast((P, LEN)))
                ps_o = [ps_pool.tile([P, NH], F32, tag="ab"[j], name=f"pso{h}_{j}") for j in range(NSPL)]
                ps_r = [ps_pool.tile([P, NH], F32, tag="r", name=f"psr{h}_{j}")[0:1, :] for j in range(NSPL)]
                egs = []
                for g in range(NG):
                    s_t = work.tile([P, G, LEN], F32, tag="s")
                    e_t = epool.tile([P, G, LEN], BF16, tag="e")
                    egs.append(e_t)
                    for i in range(G):
                        c = g * G + i
                        nc.scalar.activation(
                            s_t[:, i, :], qs_bt[:], AF.Tanh, bias=ks_t[c][:, h:h + 1])
                    nc.scalar.activation(e_t[:], s_t[:], AF.Exp)
                    for i in range(G):
                        c = g * G + i
                        lhsT = kx_t[c][:, ds(h * P, P)]
                        st, sp = c == 0, c == C - 1
                        for j in range(NSPL):
                            mm(ps_o[j][:], lhsT, e_t[:, i, ts(j, NH)], st, sp)
                        for j in range(NSPL):
                            mm(ps_r[j][:], ones_sb[:], e_t[:, i, ts(j, NH)], st, sp)
                # free psums fast (copies), then recip chain off the hot path
                oT_raw = work.tile([P, LEN], F32, tag="oraw")
                recip_f = rec_pool.tile([1, LEN], F32, tag="recipf")
                for j in range(NSPL):
                    rs_sb = rec_pool.tile([1, NH], F32, tag=f"rs{j}", name=f"rs{h}_{j}")
                    nc.vector.tensor_copy(out=rs_sb[:], in_=ps_r[j][:])
                    nc.vector.tensor_copy(out=oT_raw[:, ts(j, NH)], in_=ps_o[j][:])
                    nc.vector.reciprocal(recip_f[:, ts(j, NH)], rs_sb[:])
                recip16 = rec_pool.tile([1, LEN], BF16, tag="recip16")
                nc.vector.tensor_copy(out=recip16[:], in_=recip_f[:])
                nc.sync.dma_start(rec_dram[h:h + 1, :], recip16[:])
                recip_b = work.tile([P, LEN], BF16, tag="recipb")
                nc.sync.dma_start(recip_b[:], rec_dram[h:h + 1, :].to_broadcast((P, LEN)))
                nc.vector.tensor_tensor(
                    oT_sb[:, h, :], oT_raw[:], recip_b[:], ALU.mult)
                for g in range(NG):
                    nc.vector.tensor_tensor(
                        egs[g][:], egs[g][:],
                        recip_b[:, None, :].to_broadcast((P, G, LEN)), ALU.mult)
                    nc.sync.dma_start(
                        scoreT[h, ds(g * G * P, G * P), :].rearrange(
                            "(i p) q -> p i q", p=P),
                        egs[g][:])

            # ---- final projection: out = oT.T @ wp + bp ----
            NHo = min(512, OD)
            NSPLo = (OD + NHo - 1) // NHo
            for t in range(T):
                psf = [ps_pool.tile([P, NHo], F32, tag="ab"[j], name=f"psf{t}_{j}") for j in range(NSPLo)]
                for c in range(OC):
                    lhsT = oT_sb[:, c, ts(t, P)]
                    st, sp = c == 0, c == OC - 1
                    for j in range(NSPLo):
                        mm(psf[j][:], lhsT, wp_sb[:, c, ts(j, NHo)], st, sp)
                fin = work.tile([P, OD], F32, tag="fin")
                for j in range(NSPLo):
                    nc.vector.tensor_tensor(
                        fin[:, ts(j, NHo)], psf[j][:], bp_sb[:, ts(j, NHo)], ALU.add)
                nc.sync.dma_start(out[ts(t, P), :], fin[:])

    return nc


def host_prep(k, q, Wk, bk, Wq, bq, w, Wp, bp, H=N_HEAD, HID_=HID):
    """Per-core input maps.  Folds the per-head score vectors into the
    projections (ks = kx_h @ w_k becomes extra columns of Wk)."""
    k = np.asarray(k, np.float32)
    q = np.asarray(q, np.float32)
    Wk = np.asarray(Wk, np.float32)
    bk = np.asarray(bk, np.float32)
    Wq = np.asarray(Wq, np.float32)
    bq = np.asarray(bq, np.float32)
    w = np.asarray(w, np.float32)
    Wp = np.asarray(Wp, np.float32)
    bp = np.asarray(bp, np.float32)
    D = Wk.shape[0]
    mb = k.shape[0]

    U_k = np.einsum("dhj,j->dh", Wk.reshape(D, H, HID_), w[:HID_]).astype(np.float32)
    c_k = (bk.reshape(H, HID_) @ w[:HID_]).astype(np.float32)
    U_q = np.einsum("dhj,j->dh", Wq.reshape(D, H, HID_), w[HID_:]).astype(np.float32)
    c_q = (bq.reshape(H, HID_) @ w[HID_:]).astype(np.float32)

    bf16 = ml_dtypes.bfloat16
    shared = {
        "wk": np.ascontiguousarray(np.concatenate([Wk, U_k], axis=1)).astype(bf16),
        "bk": np.ascontiguousarray(np.concatenate([bk, c_k])),
        "uq": np.ascontiguousarray(U_q).astype(bf16), "cq": c_q,
        "wp": Wp.astype(bf16), "bp": bp,
        "ident": np.eye(128, dtype=np.float32),
    }
    in_maps = []
    for b in range(mb):
        m = dict(shared)
        m["kT"] = np.ascontiguousarray(k[b].T).astype(ml_dtypes.bfloat16)
        m["qT"] = np.ascontiguousarray(q[b].T).astype(ml_dtypes.bfloat16)
        in_maps.append(m)
    return in_maps


_NC_CACHE = {}


def _install_ntff_shim():
    """This image lacks ``antenv.axon_hooks``; recreate it and register the
    ctypes NTFF hook against the injected libaxon_pjrt.so, and skip the S3
    artifact upload (no bucket access here)."""
    import sys, types
    try:
        from antenv.axon_hooks import get_axon_ntff_profile_hook  # noqa: F401
        return
    except ImportError:
        pass
    import antenv
    mod = types.ModuleType("antenv.axon_hooks")
    _h = [None]
    mod.set_axon_ntff_profile_hook = lambda h: _h.__setitem__(0, h)
    mod.get_axon_ntff_profile_hook = lambda: _h[0]
    sys.modules["antenv.axon_hooks"] = mod
    antenv.axon_hooks = mod
    try:
        from trn_agent_boot.trn_boot import _ntff_profile_via_ctypes
        mod.set_axon_ntff_profile_hook(
            _ntff_profile_via_ctypes("/opt/axon/libaxon_pjrt.so"))
    except Exception:
        pass
    import concourse.bass_utils as bu
    bu.upload_artifacts = lambda tmpdir: str(tmpdir)


def kernel(k, q, Wk, bk, Wq, bq, w, Wp, bp, _trace=False):
    from concourse.bass_utils import run_bass_kernel_spmd

    if _trace:
        _install_ntff_shim()

    in_maps = host_prep(k, q, Wk, bk, Wq, bq, w, Wp, bp)
    if "nc" not in _NC_CACHE:
        _NC_CACHE["nc"] = build()
    nc = _NC_CACHE["nc"]
    kwargs = {}
    if _trace:
        kwargs = dict(trace=True, trace_cores=[0])
    res = run_bass_kernel_spmd(nc, in_maps, core_ids=list(range(MB)), **kwargs)

    out = np.stack([res.results[b]["out"] for b in range(MB)])
    # scoreT per core: [H, k, q]; reference wants score[h*MB+b, q, k].
    sT = np.stack([res.results[b]["scoreT"] for b in range(MB)])  # [b, h, k, q]
    score = sT.transpose(1, 0, 3, 2).reshape(N_HEAD * MB, LEN, LEN).astype(np.float32)
    if _trace:
        kernel._last_results = res
    return out, score
